# revision 29
# baseline (speedup 1.0000x reference)
"""DigitCaps (capsule routing) Trainium2 kernel, v1 (h-inner layout).

Self-contained: hardcodes shapes for
  x: [256, 32, 8, 6, 6] f32, W: [1, 10, 1152, 16, 8] f32 -> v: [256, 10, 16] f32

Sharding: pure data parallelism over batch, 32 batch items per core on 8
cores, processed as 4 octet groups per core.

Per-core layout: partition p = (i16, b8); u free dims ordered (ic=72,
w=16, h=10) with h INNERMOST so both big DVE muls run in fp16 2x mode
with no operand materialization:
  - s-pass: pr = u * c with c[p,ic,h] broadcast along w (middle axis);
  - a-pass: pr = u * vb with vb[p,w,h] broadcast along ic (outer axis).
u = W@x from block-diag packed fp16 matmuls (K=128: lhsT = host-built
block-diag x tile, rhs = repacked W, w-major/h-minor free order).
Logits are fp16, produced directly by in-place pairwise fold trees over w
(all 2x); l(t) = U.(v0+..+v_{t-1}) is recomputed fresh each iteration from
the running V so there is no read-modify-write on logits.  Softmax safety
shift: per-(b,h) max via fp16 max-fold tree over ic, a DMA xbar transpose
to fold i16 across partitions, and a tiny PE matmul (lhsT = max-bcast
view, rhs = eye40) to broadcast the per-(b,h) max back to all partitions.
The softmax denominator d = sum_i c accumulates in spare PSUM columns
(160:170) of the s-reduction tile by streaming cexp chunks through the
same sdelta matmul.  sqrt in squash is exp(0.5*ln(x)) so ACT stays on one
table set.  Output rows are (w,h)-ordered; the host transposes.
"""

import numpy as np

# ---- problem constants (hardcoded) ----
B_FULL = 256
N_CORES = 8
B_CORE = B_FULL // N_CORES          # 32
NGRP = 4                            # octet groups per core
B8 = 8                              # batch per group
H = 10
WD = 16
WH = WD * H                         # 160
S = 8
NI = 1152
I16 = 16
IC = NI // I16                      # 72
ICQ = 9                             # ic per wpack DMA chunk
XDC = 18                            # ic per xdiag DMA chunk
CPY = 3                             # ic per u psum copy tile
CKS = 24                            # ic per s-pass mul chunk
P = 128
GH = NGRP * H                       # 40

_CACHE = {}


def _build_program(debug: bool, dumps: bool = False):
    import concourse.bacc as bacc
    import concourse.bass as bass
    import concourse.tile as tile
    from concourse import mybir

    f32 = mybir.dt.float32
    f16 = mybir.dt.float16
    AX = mybir.AxisListType
    AF = mybir.ActivationFunctionType

    if not getattr(bacc, "_digitcaps_act_pin", False):
        _orig_gat = bacc.get_activation_tables

        def _pinned_gat(arch):
            tables = dict(_orig_gat(arch))
            both = {mybir.ActivationFunctionType.Exp, mybir.ActivationFunctionType.Ln}
            for name in tables:
                if name != "natural_log_exp_and_others" and both & tables[name]:
                    tables[name] = tables[name] - both
            return tables

        bacc.get_activation_tables = _pinned_gat
        bacc._digitcaps_act_pin = True

    nc = bacc.Bacc(
        "TRN2", target_bir_lowering=False, debug=debug, enable_asserts=False
    )

    xd_d = nc.dram_tensor("xdiag", [NGRP, P, IC * P], f16, kind="ExternalInput")
    w_d = nc.dram_tensor("wpack", [IC, P, WH], f16, kind="ExternalInput")
    sd_d = nc.dram_tensor("sdelta", [P, B8], f16, kind="ExternalInput")
    sr_d = nc.dram_tensor("srepl", [B8, P], f16, kind="ExternalInput")
    ey_d = nc.dram_tensor("eye40", [GH, GH], f16, kind="ExternalInput")
    out_d = nc.dram_tensor("vout", [B_CORE, WH], f32, kind="ExternalOutput")
    if dumps:
        dbg_u = nc.dram_tensor("dbg_u", [P, IC * WH], f16, kind="ExternalOutput")
        dbg_l = nc.dram_tensor("dbg_l", [P, NGRP * IC * H], f16, kind="ExternalOutput")
        dbg_m = nc.dram_tensor("dbg_m", [P, GH], f16, kind="ExternalOutput")
        dbg_c = nc.dram_tensor("dbg_c", [P, NGRP * IC * H], f16, kind="ExternalOutput")
        dbg_s = nc.dram_tensor("dbg_s", [B8, NGRP * 170], f32, kind="ExternalOutput")
        dbg_p = nc.dram_tensor("dbg_p", [P, CKS * WH], f16, kind="ExternalOutput")
        dbg_l2 = nc.dram_tensor("dbg_l2", [P, NGRP * IC * H], f16, kind="ExternalOutput")
        dbg_s2 = nc.dram_tensor("dbg_s2", [B8, NGRP * 170], f32, kind="ExternalOutput")
        dbg_v = nc.dram_tensor("dbg_v", [B8, NGRP * WH], f16, kind="ExternalOutput")

    with tile.TileContext(nc) as tc:
        with (
            tc.tile_pool(name="const", bufs=1) as const_pool,
            tc.tile_pool(name="wp", bufs=1) as wp_pool,
            tc.tile_pool(name="xd", bufs=2) as xd_pool,
            tc.tile_pool(name="u", bufs=4) as u_pool,
            tc.tile_pool(name="prs", bufs=2) as prs_pool,
            tc.tile_pool(name="pra", bufs=1) as pra_pool,
            tc.tile_pool(name="lg", bufs=1) as lg_pool,
            tc.tile_pool(name="cexp", bufs=1) as c_pool,
            tc.tile_pool(name="mx", bufs=1) as mx_pool,
            tc.tile_pool(name="small", bufs=2) as small_pool,
            tc.tile_pool(name="vv", bufs=1) as vv_pool,
            tc.tile_pool(name="psum_u", bufs=3, space="PSUM") as psum_u,
            tc.tile_pool(name="psum_s", bufs=2, space="PSUM") as psum_s,
            tc.tile_pool(name="psum_v", bufs=1, space="PSUM") as psum_v,
        ):
            sdelta = const_pool.tile([P, B8], f16, tag="sdelta")
            nc.sync.dma_start(sdelta[:], sd_d[:])
            srepl = const_pool.tile([B8, P], f16, tag="srepl")
            nc.sync.dma_start(srepl[:], sr_d[:])
            eye40 = const_pool.tile([GH, GH], f16, tag="eye40")
            nc.sync.dma_start(eye40[:], ey_d[:])

            # resident W pack, split per chunk so deps are chunk-granular
            wpq = []
            for qi, q in enumerate(range(0, IC, ICQ)):
                wq = wp_pool.tile([P, ICQ, WH], f16, tag=f"wp{qi}")
                nc.sync.dma_start(
                    wq[:], w_d[q : q + ICQ].rearrange("ic p f -> p ic f")
                )
                wpq.append(wq)

            # persistent logits [P, g, ic, h] fp16; mh bounce tile [P, 128]
            logits = lg_pool.tile([P, NGRP, IC, H], f16, tag="logits")
            mh128 = lg_pool.tile([P, P], f16, tag="mh128")
            nc.gpsimd.memset(mh128[:, GH:P], 0.0)

            # V = running sum of v (fp16), vfin = final f32 v
            V = vv_pool.tile([B8, NGRP, WD, H], f16, tag="V")
            vb16 = vv_pool.tile([P, NGRP, WD, H], f16, tag="vb16")
            sun = vv_pool.tile([B8, NGRP, 170], f32, tag="sun")

            us = []
            dump_m = [True]
            dump_p = [True]

            def ugen_and_s0(g):
                """u-gen for group g; streams s0 partial sums on the fly."""
                u = u_pool.tile([P, IC, WD, H], f16, tag="u")
                sps = psum_s.tile([B8, CPY, 170], f32, tag="sps")
                for xc in range(0, IC, XDC):
                    xd = xd_pool.tile([P, XDC, P], f16, tag="xd")
                    nc.scalar.dma_start(
                        xd[:],
                        xd_d[g].rearrange("p (ic m) -> p ic m", ic=IC)[
                            :, xc : xc + XDC
                        ],
                    )
                    for j in range(0, XDC, CPY):
                        ps = psum_u.tile([P, CPY, WH], f32, tag="ups")
                        for t in range(CPY):
                            ic = xc + j + t
                            nc.tensor.matmul(
                                ps[:, t, :],
                                xd[:, j + t, :],
                                wpq[ic // ICQ][:, ic % ICQ, :],
                                start=True,
                                stop=True,
                            )
                        ic0 = xc + j
                        nc.scalar.copy(
                            u[:, ic0 : ic0 + CPY],
                            ps[:].rearrange("p a (w h) -> p a w h", h=H),
                        )
                        # s0 partial: stream u chunk through sdelta matmul
                        nc.tensor.matmul(
                            sps[:, :, 0:WH],
                            sdelta[:],
                            u[:, ic0 : ic0 + CPY],
                            start=(ic0 == 0),
                            stop=(ic0 == IC - CPY),
                        )
                us.append(u)
                # fold the 3 partial columns -> sun[:, g]
                nc.vector.reduce_sum(
                    sun[:, g, 0:WH],
                    sps[:, :, 0:WH].rearrange("b a f -> b f a"),
                    axis=AX.X,
                )

            def squash(g_slice, it):
                """Squash sun -> v; writes V/vfin and returns v tile.
                g_slice: list of groups covered (all, fused)."""
                n = len(g_slice)
                g0 = g_slice[0]
                sw = sun[:, g0 : g0 + n, 0:WH].rearrange(
                    "b g (w h) -> b g w h", h=H
                )
                s = small_pool.tile([B8, n, WD, H], f32, tag="s")
                if it == 0:
                    nc.vector.tensor_scalar_mul(s[:], sw, 1.0 / NI)
                else:
                    dinv = small_pool.tile([B8, n, H], f32, tag="dinv")
                    nc.vector.reciprocal(
                        dinv[:],
                        sun[:, g0 : g0 + n, WH:170].rearrange(
                            "b g h -> b g h"
                        ),
                    )
                    nc.vector.tensor_mul(
                        s[:], sw, dinv[:].unsqueeze(2).to_broadcast([B8, n, WD, H])
                    )
                s2 = small_pool.tile([B8, n, WD, H], f32, tag="s2")
                nc.scalar.activation(s2[:], s[:], AF.Square)
                sq = small_pool.tile([B8, n, H], f32, tag="sq")
                nc.vector.reduce_sum(
                    sq[:], s2[:].rearrange("b g w h -> b g h w"), axis=AX.X
                )
                lgq = small_pool.tile([B8, n, H], f32, tag="lgq")
                nc.scalar.activation(lgq[:], sq[:], AF.Ln)
                rt = small_pool.tile([B8, n, H], f32, tag="rt")
                nc.scalar.activation(rt[:], lgq[:], AF.Exp, scale=0.5)
                onep = small_pool.tile([B8, n, H], f32, tag="onep")
                nc.vector.tensor_scalar_add(onep[:], sq[:], 1.0)
                rr = small_pool.tile([B8, n, H], f32, tag="rr")
                nc.vector.reciprocal(rr[:], onep[:])
                f = small_pool.tile([B8, n, H], f32, tag="f")
                nc.vector.tensor_mul(f[:], rt[:], rr[:])
                fb = f[:].unsqueeze(2).to_broadcast([B8, n, WD, H])
                if it == 2:
                    vfin = small_pool.tile([B8, n, WD, H], f32, tag="vfin")
                    nc.vector.tensor_mul(vfin[:], s[:], fb)
                    return vfin
                if it == 0:
                    # V slot(s) initialized directly
                    nc.vector.tensor_mul(V[:, g0 : g0 + n], s[:], fb)
                    return None
                v16 = small_pool.tile([B8, n, WD, H], f16, tag="v16")
                nc.vector.tensor_mul(v16[:], s[:], fb)
                nc.vector.tensor_add(V[:, g0 : g0 + n], V[:, g0 : g0 + n], v16[:])
                return None

            def vbcast(g_slice):
                """vb16[:, g] = broadcast of V[:, g] to all partitions."""
                for g in g_slice:
                    vbp = psum_v.tile([P, WD, H], f32, tag="vbp")
                    nc.tensor.matmul(
                        vbp[:], srepl[:], V[:, g], start=True, stop=True
                    )
                    nc.scalar.copy(vb16[:, g], vbp[:])

            def apass(g, fold_eng=None):
                """logits[:, g] = sum_w u * vb16[:, g] via in-place fold tree.
                fold_eng: engine for the fold adds (default DVE); GpSimd
                offload lets the folds hide behind other groups' DVE muls."""
                eng = fold_eng or nc.vector
                u = us[g]
                pra = pra_pool.tile([P, IC, WD, H], f16, tag="pra")
                vbb = vb16[:, g].unsqueeze(1).to_broadcast([P, IC, WD, H])
                nc.vector.tensor_mul(pra[:], u[:], vbb)
                eng.tensor_add(
                    pra[:, :, 0:8, :], pra[:, :, 0:8, :], pra[:, :, 8:16, :]
                )
                eng.tensor_add(
                    pra[:, :, 0:4, :], pra[:, :, 0:4, :], pra[:, :, 4:8, :]
                )
                eng.tensor_add(
                    pra[:, :, 0:2, :], pra[:, :, 0:2, :], pra[:, :, 2:4, :]
                )
                eng.tensor_add(logits[:, g], pra[:, :, 0, :], pra[:, :, 1, :])

            def maxshift_exp(cexp):
                """Fused over groups: per-(b,h) max, shift logits, exp."""
                mt = mx_pool.tile([P, NGRP, 36, H], f16, tag="mt")
                nc.vector.tensor_max(
                    mt[:], logits[:, :, 0:36, :], logits[:, :, 36:72, :]
                )
                nc.vector.tensor_max(
                    mt[:, :, 0:18, :], mt[:, :, 0:18, :], mt[:, :, 18:36, :]
                )
                nc.vector.tensor_max(
                    mt[:, :, 0:9, :], mt[:, :, 0:9, :], mt[:, :, 9:18, :]
                )
                nc.vector.reduce_max(
                    mh128[:, 0:GH].rearrange("p (g h) -> p g h", g=NGRP),
                    mt[:, :, 0:9, :].rearrange("p g i h -> p g h i"),
                    axis=AX.X,
                )
                mhT = mx_pool.tile([P, P], f16, tag="mhT")
                nc.sync.dma_start_transpose(mhT[:], mh128[:])
                # fold i16 (outer half of partition index) on 40 lanes
                mxs = mx_pool.tile([GH, 64], f16, tag="mxs")
                nc.vector.tensor_max(mxs[:], mhT[0:GH, 0:64], mhT[0:GH, 64:128])
                nc.vector.tensor_max(mxs[:, 0:32], mxs[:, 0:32], mxs[:, 32:64])
                nc.vector.tensor_max(mxs[:, 0:16], mxs[:, 0:16], mxs[:, 16:32])
                nc.vector.tensor_max(mxs[:, 0:8], mxs[:, 0:8], mxs[:, 8:16])
                # broadcast back: M16[p, (g,h)] = mxs[(g,h), p%8]
                mxb = mx_pool.tile([GH, I16, B8], f16, tag="mxb")
                nc.vector.tensor_copy(
                    mxb[:], mxs[:, 0:8].unsqueeze(1).to_broadcast([GH, I16, B8])
                )
                mps = psum_v.tile([P, GH], f32, tag="mps")
                nc.tensor.matmul(
                    mps[:],
                    mxb[:],
                    eye40[:],
                    start=True,
                    stop=True,
                )
                m16 = mx_pool.tile([P, NGRP, H], f16, tag="m16")
                nc.scalar.copy(m16[:], mps[:].rearrange("p (g h) -> p g h", g=NGRP))
                if dumps and dump_m[0]:
                    dump_m[0] = False
                    nc.sync.dma_start(
                        dbg_m[:], m16[:].rearrange("p g h -> p (g h)")
                    )
                nc.vector.tensor_sub(
                    logits[:],
                    logits[:],
                    m16[:].unsqueeze(2).to_broadcast([P, NGRP, IC, H]),
                )
                nc.scalar.activation(cexp[:], logits[:], AF.Exp)

            def spass(g, cexp):
                """sun[:, g] (incl. d in cols 160:170) from pr = u*c stream."""
                u = us[g]
                sps = psum_s.tile([B8, CPY, 170], f32, tag="sps")
                dps = psum_v.tile([B8, CPY, H], f32, tag="dps")
                for ck, c0 in enumerate(range(0, IC, CKS)):
                    pr = prs_pool.tile([P, CKS, WD, H], f16, tag="pr")
                    cb = (
                        cexp[:, g, c0 : c0 + CKS, :]
                        .unsqueeze(2)
                        .to_broadcast([P, CKS, WD, H])
                    )
                    nc.vector.tensor_mul(pr[:], u[:, c0 : c0 + CKS], cb)
                    if dumps and dump_p[0] and g == 0 and c0 == 0:
                        dump_p[0] = False
                        nc.sync.dma_start(
                            dbg_p[:], pr[:].rearrange("p ic w h -> p (ic w h)")
                        )
                    for j in range(0, CKS, CPY):
                        ic = c0 + j
                        nc.tensor.matmul(
                            sps[:, :, 0:WH],
                            sdelta[:],
                            pr[:, j : j + CPY],
                            start=(ic == 0),
                            stop=(ic == IC - CPY),
                        )
                        nc.tensor.matmul(
                            dps[:],
                            sdelta[:],
                            cexp[:, g, ic : ic + CPY, :],
                            start=(ic == 0),
                            stop=(ic == IC - CPY),
                        )
                nc.vector.reduce_sum(
                    sun[:, g, 0:WH],
                    sps[:, :, 0:WH].rearrange("b a f -> b f a"),
                    axis=AX.X,
                )
                nc.vector.reduce_sum(
                    sun[:, g, WH:170],
                    dps[:].rearrange("b a h -> b h a"),
                    axis=AX.X,
                )

            # ================= iteration 0 =================
            for g in range(NGRP):
                ugen_and_s0(g)
                squash([g], 0)
                vbcast([g])
                apass(g)

            if dumps:
                nc.sync.dma_start(
                    dbg_u[:], us[0][:].rearrange("p ic w h -> p (ic w h)")
                )
                nc.sync.dma_start(
                    dbg_l[:], logits[:].rearrange("p g ic h -> p (g ic h)")
                )

            # ================= iterations 1, 2 =================
            for it in (1, 2):
                cexp = c_pool.tile([P, NGRP, IC, H], f16, tag="cexp")
                maxshift_exp(cexp)
                if dumps and it == 1:
                    nc.sync.dma_start(
                        dbg_c[:], cexp[:].rearrange("p g ic h -> p (g ic h)")
                    )
                for g in range(NGRP):
                    spass(g, cexp)
                    if it == 2:
                        vfin = squash([g], 2)
                        nc.sync.dma_start(
                            out_d[g * B8 : (g + 1) * B8, :],
                            vfin[:].rearrange("b g w h -> b (g w h)"),
                        )
                if dumps and it == 1:
                    nc.sync.dma_start(
                        dbg_s[:], sun[:].rearrange("b g f -> b (g f)")
                    )
                if it == 1:
                    squash(list(range(NGRP)), 1)
                    if dumps:
                        nc.sync.dma_start(
                            dbg_v[:], V[:].rearrange("b g w h -> b (g w h)")
                        )
                    vbcast(list(range(NGRP)))
                    for g in range(NGRP):
                        apass(g)
                    if dumps:
                        nc.sync.dma_start(
                            dbg_l2[:], logits[:].rearrange("p g ic h -> p (g ic h)")
                        )
                elif dumps:
                    nc.sync.dma_start(
                        dbg_s2[:], sun[:].rearrange("b g f -> b (g f)")
                    )

    nc.compile()
    return nc


def _host_inputs(x: np.ndarray, W: np.ndarray):
    """Build per-core input maps."""
    xr = np.ascontiguousarray(x.reshape(B_FULL, NI, S).astype(np.float32, copy=False))
    W0 = np.asarray(W, dtype=np.float32).reshape(H, NI, WD, S)
    # wpack[ic, (i16,s), (w,h)] = W0[h, ic*16+i16, w, s]
    wpack = np.ascontiguousarray(
        W0.reshape(H, IC, I16, WD, S)
        .transpose(1, 2, 4, 3, 0)
        .reshape(IC, P, WH)
        .astype(np.float16)
    )
    # sdelta[p, b'] = (p % 8 == b');  srepl = sdelta.T
    pidx = np.arange(P)
    sdelta = (pidx[:, None] % B8 == np.arange(B8)[None, :]).astype(np.float16)
    srepl = np.ascontiguousarray(sdelta.T)
    eye40 = np.eye(GH, dtype=np.float16)

    in_maps = []
    for c in range(N_CORES):
        xc = xr[c * B_CORE : (c + 1) * B_CORE]  # [32, 1152, 8]
        # xdiag[g, (i16,s), ic*128 + i16*8 + b] = xc[g*8+b, ic*16+i16, s]
        xd = np.zeros((NGRP, P, IC, I16, B8), dtype=np.float16)
        xg = xc.reshape(NGRP, B8, IC, I16, S).astype(np.float16)
        for k in range(I16):
            xd[:, k * S : (k + 1) * S, :, k, :] = xg[:, :, :, k, :].transpose(
                0, 3, 2, 1
            )
        in_maps.append(
            {
                "xdiag": np.ascontiguousarray(xd.reshape(NGRP, P, IC * P)),
                "wpack": wpack,
                "sdelta": sdelta,
                "srepl": srepl,
                "eye40": eye40,
            }
        )
    return in_maps


def _unshard(vout: np.ndarray) -> np.ndarray:
    """Per-core vout [B_CORE, (w,h)] -> [B_CORE, H, WD]."""
    return vout.reshape(B_CORE, WD, H).transpose(0, 2, 1)


def kernel(x: np.ndarray, W: np.ndarray) -> np.ndarray:
    from concourse import bass_utils

    if "nc" not in _CACHE:
        _CACHE["nc"] = _build_program(debug=False)
    nc = _CACHE["nc"]
    in_maps = _host_inputs(x, W)
    res = bass_utils.run_bass_kernel_spmd(nc, in_maps, list(range(N_CORES)))
    outs = [_unshard(res.results[c]["vout"]) for c in range(N_CORES)]
    return np.concatenate(outs, axis=0).astype(np.float32)


# revision 31
# speedup vs baseline: 1.0942x; 1.0942x over previous
"""DigitCaps (capsule routing) Trainium2 kernel, v1 (h-inner layout).

Self-contained: hardcodes shapes for
  x: [256, 32, 8, 6, 6] f32, W: [1, 10, 1152, 16, 8] f32 -> v: [256, 10, 16] f32

Sharding: pure data parallelism over batch, 32 batch items per core on 8
cores, processed as 4 octet groups per core.

Per-core layout: partition p = (i16, b8); u free dims ordered (ic=72,
w=16, h=10) with h INNERMOST so both big DVE muls run in fp16 2x mode
with no operand materialization:
  - s-pass: pr = u * c with c[p,ic,h] broadcast along w (middle axis);
  - a-pass: pr = u * vb with vb[p,w,h] broadcast along ic (outer axis).
u = W@x from block-diag packed fp16 matmuls (K=128: lhsT = host-built
block-diag x tile, rhs = repacked W, w-major/h-minor free order).
Logits are fp16, produced directly by in-place pairwise fold trees over w
(all 2x); l(t) = U.(v0+..+v_{t-1}) is recomputed fresh each iteration from
the running V so there is no read-modify-write on logits.  Softmax safety
shift: per-(b,h) max via fp16 max-fold tree over ic, a DMA xbar transpose
to fold i16 across partitions, and a tiny PE matmul (lhsT = max-bcast
view, rhs = eye40) to broadcast the per-(b,h) max back to all partitions.
The softmax denominator d = sum_i c accumulates in spare PSUM columns
(160:170) of the s-reduction tile by streaming cexp chunks through the
same sdelta matmul.  sqrt in squash is exp(0.5*ln(x)) so ACT stays on one
table set.  Output rows are (w,h)-ordered; the host transposes.
"""

import numpy as np

# ---- problem constants (hardcoded) ----
B_FULL = 256
N_CORES = 8
B_CORE = B_FULL // N_CORES          # 32
NGRP = 4                            # octet groups per core
B8 = 8                              # batch per group
H = 10
WD = 16
WH = WD * H                         # 160
S = 8
NI = 1152
I16 = 16
IC = NI // I16                      # 72
ICQ = 9                             # ic per wpack DMA chunk
XDC = 18                            # ic per xdiag DMA chunk
CPY = 3                             # ic per u psum copy tile
CKS = 24                            # ic per s-pass mul chunk
P = 128
GH = NGRP * H                       # 40

_CACHE = {}


def _build_program(debug: bool, dumps: bool = False):
    import concourse.bacc as bacc
    import concourse.bass as bass
    import concourse.tile as tile
    from concourse import mybir

    f32 = mybir.dt.float32
    f16 = mybir.dt.float16
    AX = mybir.AxisListType
    AF = mybir.ActivationFunctionType

    if not getattr(bacc, "_digitcaps_act_pin", False):
        _orig_gat = bacc.get_activation_tables

        def _pinned_gat(arch):
            tables = dict(_orig_gat(arch))
            both = {mybir.ActivationFunctionType.Exp, mybir.ActivationFunctionType.Ln}
            for name in tables:
                if name != "natural_log_exp_and_others" and both & tables[name]:
                    tables[name] = tables[name] - both
            return tables

        bacc.get_activation_tables = _pinned_gat
        bacc._digitcaps_act_pin = True

    nc = bacc.Bacc(
        "TRN2", target_bir_lowering=False, debug=debug, enable_asserts=False
    )

    xd_d = nc.dram_tensor("xdiag", [NGRP, P, IC * P], f16, kind="ExternalInput")
    w_d = nc.dram_tensor("wpack", [IC, P, WH], f16, kind="ExternalInput")
    sd_d = nc.dram_tensor("sdelta", [P, B8], f16, kind="ExternalInput")
    sr_d = nc.dram_tensor("srepl", [B8, P], f16, kind="ExternalInput")
    ey_d = nc.dram_tensor("eye40", [GH, GH], f16, kind="ExternalInput")
    out_d = nc.dram_tensor("vout", [B_CORE, WH], f32, kind="ExternalOutput")
    if dumps:
        dbg_u = nc.dram_tensor("dbg_u", [P, IC * WH], f16, kind="ExternalOutput")
        dbg_l = nc.dram_tensor("dbg_l", [P, NGRP * IC * H], f16, kind="ExternalOutput")
        dbg_m = nc.dram_tensor("dbg_m", [P, GH], f16, kind="ExternalOutput")
        dbg_c = nc.dram_tensor("dbg_c", [P, NGRP * IC * H], f16, kind="ExternalOutput")
        dbg_s = nc.dram_tensor("dbg_s", [B8, NGRP * 170], f32, kind="ExternalOutput")
        dbg_p = nc.dram_tensor("dbg_p", [P, CKS * WH], f16, kind="ExternalOutput")
        dbg_l2 = nc.dram_tensor("dbg_l2", [P, NGRP * IC * H], f16, kind="ExternalOutput")
        dbg_s2 = nc.dram_tensor("dbg_s2", [B8, NGRP * 170], f32, kind="ExternalOutput")
        dbg_v = nc.dram_tensor("dbg_v", [B8, NGRP * WH], f16, kind="ExternalOutput")

    with tile.TileContext(nc) as tc:
        with (
            tc.tile_pool(name="const", bufs=1) as const_pool,
            tc.tile_pool(name="wp", bufs=1) as wp_pool,
            tc.tile_pool(name="xd", bufs=2) as xd_pool,
            tc.tile_pool(name="u", bufs=4) as u_pool,
            tc.tile_pool(name="prs", bufs=2) as prs_pool,
            tc.tile_pool(name="pra", bufs=1) as pra_pool,
            tc.tile_pool(name="lg", bufs=1) as lg_pool,
            tc.tile_pool(name="cexp", bufs=1) as c_pool,
            tc.tile_pool(name="mx", bufs=1) as mx_pool,
            tc.tile_pool(name="small", bufs=2) as small_pool,
            tc.tile_pool(name="vv", bufs=1) as vv_pool,
            tc.tile_pool(name="psum_u", bufs=3, space="PSUM") as psum_u,
            tc.tile_pool(name="psum_s", bufs=2, space="PSUM") as psum_s,
            tc.tile_pool(name="psum_v", bufs=1, space="PSUM") as psum_v,
        ):
            sdelta = const_pool.tile([P, B8], f16, tag="sdelta")
            nc.sync.dma_start(sdelta[:], sd_d[:])
            srepl = const_pool.tile([B8, P], f16, tag="srepl")
            nc.sync.dma_start(srepl[:], sr_d[:])
            eye40 = const_pool.tile([GH, GH], f16, tag="eye40")
            nc.sync.dma_start(eye40[:], ey_d[:])

            def load_xd(g, xc):
                xd = xd_pool.tile([P, XDC, P], f16, tag="xd")
                nc.sync.dma_start(
                    xd[:],
                    xd_d[g].rearrange("p (ic m) -> p ic m", ic=IC)[
                        :, xc : xc + XDC
                    ],
                )
                return xd

            # resident W pack, split per chunk so deps are chunk-granular.
            # Emission interleaves group 0's xd loads between wpack chunks so
            # the first u-gen matmul isn't stuck behind the whole W transfer
            # on the serial sync DMA queue.
            wpq = [None] * (IC // ICQ)
            xds0 = []

            def load_wp(qi):
                wq = wp_pool.tile([P, ICQ, WH], f16, tag=f"wp{qi}")
                nc.sync.dma_start(
                    wq[:],
                    w_d[qi * ICQ : (qi + 1) * ICQ].rearrange("ic p f -> p ic f"),
                )
                wpq[qi] = wq

            load_wp(0)
            xds0.append(load_xd(0, 0))
            load_wp(1)
            load_wp(2)
            xds0.append(load_xd(0, XDC))
            load_wp(3)
            load_wp(4)
            xds0.append(load_xd(0, 2 * XDC))
            load_wp(5)
            load_wp(6)
            xds0.append(load_xd(0, 3 * XDC))
            load_wp(7)

            # persistent logits [P, g, ic, h] fp16; mh bounce tile [P, 128]
            logits = lg_pool.tile([P, NGRP, IC, H], f16, tag="logits")
            mh128 = lg_pool.tile([P, P], f16, tag="mh128")
            nc.gpsimd.memset(mh128[:, GH:P], 0.0)

            # V = running sum of v (fp16), vfin = final f32 v
            V = vv_pool.tile([B8, NGRP, WD, H], f16, tag="V")
            vb16 = vv_pool.tile([P, NGRP, WD, H], f16, tag="vb16")
            sun = vv_pool.tile([B8, NGRP, 170], f32, tag="sun")

            us = []
            dump_m = [True]
            dump_p = [True]

            def ugen_and_s0(g):
                """u-gen for group g; streams s0 partial sums on the fly."""
                u = u_pool.tile([P, IC, WD, H], f16, tag="u")
                sps = psum_s.tile([B8, CPY, 170], f32, tag="sps")
                for xi, xc in enumerate(range(0, IC, XDC)):
                    xd = xds0[xi] if g == 0 else load_xd(g, xc)
                    for j in range(0, XDC, CPY):
                        ps = psum_u.tile([P, CPY, WH], f32, tag="ups")
                        for t in range(CPY):
                            ic = xc + j + t
                            nc.tensor.matmul(
                                ps[:, t, :],
                                xd[:, j + t, :],
                                wpq[ic // ICQ][:, ic % ICQ, :],
                                start=True,
                                stop=True,
                            )
                        ic0 = xc + j
                        nc.scalar.copy(
                            u[:, ic0 : ic0 + CPY],
                            ps[:].rearrange("p a (w h) -> p a w h", h=H),
                        )
                        # s0 partial: stream u chunk through sdelta matmul
                        nc.tensor.matmul(
                            sps[:, :, 0:WH],
                            sdelta[:],
                            u[:, ic0 : ic0 + CPY],
                            start=(ic0 == 0),
                            stop=(ic0 == IC - CPY),
                        )
                us.append(u)
                # fold the 3 partial columns -> sun[:, g]
                nc.vector.reduce_sum(
                    sun[:, g, 0:WH],
                    sps[:, :, 0:WH].rearrange("b a f -> b f a"),
                    axis=AX.X,
                )

            def squash(g_slice, it):
                """Squash sun -> v; writes V/vfin and returns v tile.
                g_slice: list of groups covered (all, fused)."""
                n = len(g_slice)
                g0 = g_slice[0]
                sw = sun[:, g0 : g0 + n, 0:WH].rearrange(
                    "b g (w h) -> b g w h", h=H
                )
                s = small_pool.tile([B8, n, WD, H], f32, tag="s")
                if it == 0:
                    nc.vector.tensor_scalar_mul(s[:], sw, 1.0 / NI)
                else:
                    dinv = small_pool.tile([B8, n, H], f32, tag="dinv")
                    nc.vector.reciprocal(
                        dinv[:],
                        sun[:, g0 : g0 + n, WH:170].rearrange(
                            "b g h -> b g h"
                        ),
                    )
                    nc.vector.tensor_mul(
                        s[:], sw, dinv[:].unsqueeze(2).to_broadcast([B8, n, WD, H])
                    )
                s2 = small_pool.tile([B8, n, WD, H], f32, tag="s2")
                nc.scalar.activation(s2[:], s[:], AF.Square)
                sq = small_pool.tile([B8, n, H], f32, tag="sq")
                nc.vector.reduce_sum(
                    sq[:], s2[:].rearrange("b g w h -> b g h w"), axis=AX.X
                )
                lgq = small_pool.tile([B8, n, H], f32, tag="lgq")
                nc.scalar.activation(lgq[:], sq[:], AF.Ln)
                rt = small_pool.tile([B8, n, H], f32, tag="rt")
                nc.scalar.activation(rt[:], lgq[:], AF.Exp, scale=0.5)
                onep = small_pool.tile([B8, n, H], f32, tag="onep")
                nc.vector.tensor_scalar_add(onep[:], sq[:], 1.0)
                rr = small_pool.tile([B8, n, H], f32, tag="rr")
                nc.vector.reciprocal(rr[:], onep[:])
                f = small_pool.tile([B8, n, H], f32, tag="f")
                nc.vector.tensor_mul(f[:], rt[:], rr[:])
                fb = f[:].unsqueeze(2).to_broadcast([B8, n, WD, H])
                if it == 2:
                    vfin = small_pool.tile([B8, n, WD, H], f32, tag="vfin")
                    nc.vector.tensor_mul(vfin[:], s[:], fb)
                    return vfin
                if it == 0:
                    # V slot(s) initialized directly
                    nc.vector.tensor_mul(V[:, g0 : g0 + n], s[:], fb)
                    return None
                v16 = small_pool.tile([B8, n, WD, H], f16, tag="v16")
                nc.vector.tensor_mul(v16[:], s[:], fb)
                nc.vector.tensor_add(V[:, g0 : g0 + n], V[:, g0 : g0 + n], v16[:])
                return None

            def vbcast(g_slice):
                """vb16[:, g] = broadcast of V[:, g] to all partitions."""
                for g in g_slice:
                    vbp = psum_v.tile([P, WD, H], f32, tag="vbp")
                    nc.tensor.matmul(
                        vbp[:], srepl[:], V[:, g], start=True, stop=True
                    )
                    nc.scalar.copy(vb16[:, g], vbp[:])

            def apass(g, fold_eng=None):
                """logits[:, g] = sum_w u * vb16[:, g] via in-place fold tree.
                fold_eng: engine for the fold adds (default DVE); GpSimd
                offload lets the folds hide behind other groups' DVE muls."""
                eng = fold_eng or nc.vector
                u = us[g]
                pra = pra_pool.tile([P, IC, WD, H], f16, tag="pra")
                vbb = vb16[:, g].unsqueeze(1).to_broadcast([P, IC, WD, H])
                nc.vector.tensor_mul(pra[:], u[:], vbb)
                eng.tensor_add(
                    pra[:, :, 0:8, :], pra[:, :, 0:8, :], pra[:, :, 8:16, :]
                )
                eng.tensor_add(
                    pra[:, :, 0:4, :], pra[:, :, 0:4, :], pra[:, :, 4:8, :]
                )
                eng.tensor_add(
                    pra[:, :, 0:2, :], pra[:, :, 0:2, :], pra[:, :, 2:4, :]
                )
                eng.tensor_add(logits[:, g], pra[:, :, 0, :], pra[:, :, 1, :])

            def maxshift_exp(cexp):
                """Fused over groups: per-(b,h) max, shift logits, exp."""
                mt = mx_pool.tile([P, NGRP, 36, H], f16, tag="mt")
                nc.vector.tensor_max(
                    mt[:], logits[:, :, 0:36, :], logits[:, :, 36:72, :]
                )
                nc.vector.tensor_max(
                    mt[:, :, 0:18, :], mt[:, :, 0:18, :], mt[:, :, 18:36, :]
                )
                nc.vector.tensor_max(
                    mt[:, :, 0:9, :], mt[:, :, 0:9, :], mt[:, :, 9:18, :]
                )
                nc.vector.reduce_max(
                    mh128[:, 0:GH].rearrange("p (g h) -> p g h", g=NGRP),
                    mt[:, :, 0:9, :].rearrange("p g i h -> p g h i"),
                    axis=AX.X,
                )
                mhT = mx_pool.tile([P, P], f16, tag="mhT")
                nc.sync.dma_start_transpose(mhT[:], mh128[:])
                # fold i16 (outer half of partition index) on 40 lanes
                mxs = mx_pool.tile([GH, 64], f16, tag="mxs")
                nc.vector.tensor_max(mxs[:], mhT[0:GH, 0:64], mhT[0:GH, 64:128])
                nc.vector.tensor_max(mxs[:, 0:32], mxs[:, 0:32], mxs[:, 32:64])
                nc.vector.tensor_max(mxs[:, 0:16], mxs[:, 0:16], mxs[:, 16:32])
                nc.vector.tensor_max(mxs[:, 0:8], mxs[:, 0:8], mxs[:, 8:16])
                # broadcast back: M16[p, (g,h)] = mxs[(g,h), p%8]
                mxb = mx_pool.tile([GH, I16, B8], f16, tag="mxb")
                nc.vector.tensor_copy(
                    mxb[:], mxs[:, 0:8].unsqueeze(1).to_broadcast([GH, I16, B8])
                )
                mps = psum_v.tile([P, GH], f32, tag="mps")
                nc.tensor.matmul(
                    mps[:],
                    mxb[:],
                    eye40[:],
                    start=True,
                    stop=True,
                )
                m16 = mx_pool.tile([P, NGRP, H], f16, tag="m16")
                nc.scalar.copy(m16[:], mps[:].rearrange("p (g h) -> p g h", g=NGRP))
                if dumps and dump_m[0]:
                    dump_m[0] = False
                    nc.sync.dma_start(
                        dbg_m[:], m16[:].rearrange("p g h -> p (g h)")
                    )
                nc.vector.tensor_sub(
                    logits[:],
                    logits[:],
                    m16[:].unsqueeze(2).to_broadcast([P, NGRP, IC, H]),
                )
                nc.scalar.activation(cexp[:], logits[:], AF.Exp)

            def spass(g, cexp):
                """sun[:, g] (incl. d in cols 160:170) from pr = u*c stream."""
                u = us[g]
                sps = psum_s.tile([B8, CPY, 170], f32, tag="sps")
                dps = psum_v.tile([B8, CPY, H], f32, tag="dps")
                for ck, c0 in enumerate(range(0, IC, CKS)):
                    pr = prs_pool.tile([P, CKS, WD, H], f16, tag="pr")
                    cb = (
                        cexp[:, g, c0 : c0 + CKS, :]
                        .unsqueeze(2)
                        .to_broadcast([P, CKS, WD, H])
                    )
                    nc.vector.tensor_mul(pr[:], u[:, c0 : c0 + CKS], cb)
                    if dumps and dump_p[0] and g == 0 and c0 == 0:
                        dump_p[0] = False
                        nc.sync.dma_start(
                            dbg_p[:], pr[:].rearrange("p ic w h -> p (ic w h)")
                        )
                    for j in range(0, CKS, CPY):
                        ic = c0 + j
                        nc.tensor.matmul(
                            sps[:, :, 0:WH],
                            sdelta[:],
                            pr[:, j : j + CPY],
                            start=(ic == 0),
                            stop=(ic == IC - CPY),
                        )
                        nc.tensor.matmul(
                            dps[:],
                            sdelta[:],
                            cexp[:, g, ic : ic + CPY, :],
                            start=(ic == 0),
                            stop=(ic == IC - CPY),
                        )
                nc.vector.reduce_sum(
                    sun[:, g, 0:WH],
                    sps[:, :, 0:WH].rearrange("b a f -> b f a"),
                    axis=AX.X,
                )
                nc.vector.reduce_sum(
                    sun[:, g, WH:170],
                    dps[:].rearrange("b a h -> b h a"),
                    axis=AX.X,
                )

            # ================= iteration 0 =================
            for g in range(NGRP):
                ugen_and_s0(g)
                squash([g], 0)
                vbcast([g])
                apass(g)

            if dumps:
                nc.sync.dma_start(
                    dbg_u[:], us[0][:].rearrange("p ic w h -> p (ic w h)")
                )
                nc.sync.dma_start(
                    dbg_l[:], logits[:].rearrange("p g ic h -> p (g ic h)")
                )

            # ================= iterations 1, 2 =================
            for it in (1, 2):
                cexp = c_pool.tile([P, NGRP, IC, H], f16, tag="cexp")
                maxshift_exp(cexp)
                if dumps and it == 1:
                    nc.sync.dma_start(
                        dbg_c[:], cexp[:].rearrange("p g ic h -> p (g ic h)")
                    )
                for g in range(NGRP):
                    spass(g, cexp)
                    if it == 2:
                        vfin = squash([g], 2)
                        nc.sync.dma_start(
                            out_d[g * B8 : (g + 1) * B8, :],
                            vfin[:].rearrange("b g w h -> b (g w h)"),
                        )
                if dumps and it == 1:
                    nc.sync.dma_start(
                        dbg_s[:], sun[:].rearrange("b g f -> b (g f)")
                    )
                if it == 1:
                    squash(list(range(NGRP)), 1)
                    if dumps:
                        nc.sync.dma_start(
                            dbg_v[:], V[:].rearrange("b g w h -> b (g w h)")
                        )
                    vbcast(list(range(NGRP)))
                    for g in range(NGRP):
                        apass(g)
                    if dumps:
                        nc.sync.dma_start(
                            dbg_l2[:], logits[:].rearrange("p g ic h -> p (g ic h)")
                        )
                elif dumps:
                    nc.sync.dma_start(
                        dbg_s2[:], sun[:].rearrange("b g f -> b (g f)")
                    )

    nc.compile()
    return nc


def _host_inputs(x: np.ndarray, W: np.ndarray):
    """Build per-core input maps."""
    xr = np.ascontiguousarray(x.reshape(B_FULL, NI, S).astype(np.float32, copy=False))
    W0 = np.asarray(W, dtype=np.float32).reshape(H, NI, WD, S)
    # wpack[ic, (i16,s), (w,h)] = W0[h, ic*16+i16, w, s]
    wpack = np.ascontiguousarray(
        W0.reshape(H, IC, I16, WD, S)
        .transpose(1, 2, 4, 3, 0)
        .reshape(IC, P, WH)
        .astype(np.float16)
    )
    # sdelta[p, b'] = (p % 8 == b');  srepl = sdelta.T
    pidx = np.arange(P)
    sdelta = (pidx[:, None] % B8 == np.arange(B8)[None, :]).astype(np.float16)
    srepl = np.ascontiguousarray(sdelta.T)
    eye40 = np.eye(GH, dtype=np.float16)

    in_maps = []
    for c in range(N_CORES):
        xc = xr[c * B_CORE : (c + 1) * B_CORE]  # [32, 1152, 8]
        # xdiag[g, (i16,s), ic*128 + i16*8 + b] = xc[g*8+b, ic*16+i16, s]
        xd = np.zeros((NGRP, P, IC, I16, B8), dtype=np.float16)
        xg = xc.reshape(NGRP, B8, IC, I16, S).astype(np.float16)
        for k in range(I16):
            xd[:, k * S : (k + 1) * S, :, k, :] = xg[:, :, :, k, :].transpose(
                0, 3, 2, 1
            )
        in_maps.append(
            {
                "xdiag": np.ascontiguousarray(xd.reshape(NGRP, P, IC * P)),
                "wpack": wpack,
                "sdelta": sdelta,
                "srepl": srepl,
                "eye40": eye40,
            }
        )
    return in_maps


def _unshard(vout: np.ndarray) -> np.ndarray:
    """Per-core vout [B_CORE, (w,h)] -> [B_CORE, H, WD]."""
    return vout.reshape(B_CORE, WD, H).transpose(0, 2, 1)


def kernel(x: np.ndarray, W: np.ndarray) -> np.ndarray:
    from concourse import bass_utils

    if "nc" not in _CACHE:
        _CACHE["nc"] = _build_program(debug=False)
    nc = _CACHE["nc"]
    in_maps = _host_inputs(x, W)
    res = bass_utils.run_bass_kernel_spmd(nc, in_maps, list(range(N_CORES)))
    outs = [_unshard(res.results[c]["vout"]) for c in range(N_CORES)]
    return np.concatenate(outs, axis=0).astype(np.float32)


# revision 34
# speedup vs baseline: 1.1315x; 1.0341x over previous
"""DigitCaps (capsule routing) Trainium2 kernel, v1 (h-inner layout).

Self-contained: hardcodes shapes for
  x: [256, 32, 8, 6, 6] f32, W: [1, 10, 1152, 16, 8] f32 -> v: [256, 10, 16] f32

Sharding: pure data parallelism over batch, 32 batch items per core on 8
cores, processed as 4 octet groups per core.

Per-core layout: partition p = (i16, b8); u free dims ordered (ic=72,
w=16, h=10) with h INNERMOST so both big DVE muls run in fp16 2x mode
with no operand materialization:
  - s-pass: pr = u * c with c[p,ic,h] broadcast along w (middle axis);
  - a-pass: pr = u * vb with vb[p,w,h] broadcast along ic (outer axis).
u = W@x from block-diag packed fp16 matmuls (K=128: lhsT = host-built
block-diag x tile, rhs = repacked W, w-major/h-minor free order).
Logits are fp16, produced directly by in-place pairwise fold trees over w
(all 2x); l(t) = U.(v0+..+v_{t-1}) is recomputed fresh each iteration from
the running V so there is no read-modify-write on logits.  Softmax safety
shift: per-(b,h) max via fp16 max-fold tree over ic, a DMA xbar transpose
to fold i16 across partitions, and a tiny PE matmul (lhsT = max-bcast
view, rhs = eye40) to broadcast the per-(b,h) max back to all partitions.
The softmax denominator d = sum_i c accumulates in spare PSUM columns
(160:170) of the s-reduction tile by streaming cexp chunks through the
same sdelta matmul.  sqrt in squash is exp(0.5*ln(x)) so ACT stays on one
table set.  Output rows are (w,h)-ordered; the host transposes.
"""

import numpy as np

# ---- problem constants (hardcoded) ----
B_FULL = 256
N_CORES = 8
B_CORE = B_FULL // N_CORES          # 32
NGRP = 4                            # octet groups per core
B8 = 8                              # batch per group
H = 10
WD = 16
WH = WD * H                         # 160
S = 8
NI = 1152
I16 = 16
IC = NI // I16                      # 72
ICQ = 9                             # ic per wpack DMA chunk
XDC = 18                            # ic per xdiag DMA chunk
CPY = 3                             # ic per u psum copy tile
CKS = 24                            # ic per s-pass mul chunk
P = 128
GH = NGRP * H                       # 40

_CACHE = {}


def _build_program(debug: bool, dumps: bool = False):
    import concourse.bacc as bacc
    import concourse.bass as bass
    import concourse.tile as tile
    from concourse import mybir

    f32 = mybir.dt.float32
    f16 = mybir.dt.float16
    AX = mybir.AxisListType
    AF = mybir.ActivationFunctionType

    if not getattr(bacc, "_digitcaps_act_pin", False):
        _orig_gat = bacc.get_activation_tables

        def _pinned_gat(arch):
            tables = dict(_orig_gat(arch))
            both = {mybir.ActivationFunctionType.Exp, mybir.ActivationFunctionType.Ln}
            for name in tables:
                if name != "natural_log_exp_and_others" and both & tables[name]:
                    tables[name] = tables[name] - both
            return tables

        bacc.get_activation_tables = _pinned_gat
        bacc._digitcaps_act_pin = True

    nc = bacc.Bacc(
        "TRN2", target_bir_lowering=False, debug=debug, enable_asserts=False
    )

    xd_d = nc.dram_tensor("xdiag", [NGRP, P, IC * P], f16, kind="ExternalInput")
    w_d = nc.dram_tensor("wpack", [IC, P, WH], f16, kind="ExternalInput")
    sd_d = nc.dram_tensor("sdelta", [P, B8], f16, kind="ExternalInput")
    sr_d = nc.dram_tensor("srepl", [B8, P], f16, kind="ExternalInput")
    ey_d = nc.dram_tensor("eye40", [GH, GH], f16, kind="ExternalInput")
    id_d = nc.dram_tensor("ident", [P, P], f16, kind="ExternalInput")
    out_d = nc.dram_tensor("vout", [B_CORE, WH], f32, kind="ExternalOutput")
    if dumps:
        dbg_u = nc.dram_tensor("dbg_u", [P, IC * WH], f16, kind="ExternalOutput")
        dbg_l = nc.dram_tensor("dbg_l", [P, NGRP * IC * H], f16, kind="ExternalOutput")
        dbg_m = nc.dram_tensor("dbg_m", [P, GH], f16, kind="ExternalOutput")
        dbg_c = nc.dram_tensor("dbg_c", [P, NGRP * IC * H], f16, kind="ExternalOutput")
        dbg_s = nc.dram_tensor("dbg_s", [B8, NGRP * 170], f32, kind="ExternalOutput")
        dbg_p = nc.dram_tensor("dbg_p", [P, CKS * WH], f16, kind="ExternalOutput")
        dbg_l2 = nc.dram_tensor("dbg_l2", [P, NGRP * IC * H], f16, kind="ExternalOutput")
        dbg_s2 = nc.dram_tensor("dbg_s2", [B8, NGRP * 170], f32, kind="ExternalOutput")
        dbg_v = nc.dram_tensor("dbg_v", [B8, NGRP * WH], f16, kind="ExternalOutput")

    with tile.TileContext(nc) as tc:
        with (
            tc.tile_pool(name="const", bufs=1) as const_pool,
            tc.tile_pool(name="wp", bufs=1) as wp_pool,
            tc.tile_pool(name="xd", bufs=2) as xd_pool,
            tc.tile_pool(name="u", bufs=4) as u_pool,
            tc.tile_pool(name="prs", bufs=2) as prs_pool,
            tc.tile_pool(name="pra", bufs=2) as pra_pool,
            tc.tile_pool(name="lg", bufs=1) as lg_pool,
            tc.tile_pool(name="cexp", bufs=1) as c_pool,
            tc.tile_pool(name="mx", bufs=1) as mx_pool,
            tc.tile_pool(name="small", bufs=2) as small_pool,
            tc.tile_pool(name="vv", bufs=1) as vv_pool,
            tc.tile_pool(name="psum_u", bufs=2, space="PSUM") as psum_u,
            tc.tile_pool(name="psum_s", bufs=2, space="PSUM") as psum_s,
            tc.tile_pool(name="psum_v", bufs=1, space="PSUM") as psum_v,
            tc.tile_pool(name="psum_a", bufs=1, space="PSUM") as psum_a,
        ):
            sdelta = const_pool.tile([P, B8], f16, tag="sdelta")
            nc.sync.dma_start(sdelta[:], sd_d[:])
            srepl = const_pool.tile([B8, P], f16, tag="srepl")
            nc.sync.dma_start(srepl[:], sr_d[:])
            eye40 = const_pool.tile([GH, GH], f16, tag="eye40")
            nc.sync.dma_start(eye40[:], ey_d[:])
            ident = const_pool.tile([P, P], f16, tag="ident")
            nc.sync.dma_start(ident[:], id_d[:])

            def load_xd(g, xc):
                xd = xd_pool.tile([P, XDC, P], f16, tag="xd")
                nc.sync.dma_start(
                    xd[:],
                    xd_d[g].rearrange("p (ic m) -> p ic m", ic=IC)[
                        :, xc : xc + XDC
                    ],
                )
                return xd

            # resident W pack, split per chunk so deps are chunk-granular.
            # Emission interleaves group 0's xd loads between wpack chunks so
            # the first u-gen matmul isn't stuck behind the whole W transfer
            # on the serial sync DMA queue.
            wpq = [None] * (IC // ICQ)
            xds0 = []

            def load_wp(qi):
                wq = wp_pool.tile([P, ICQ, WH], f16, tag=f"wp{qi}")
                nc.sync.dma_start(
                    wq[:],
                    w_d[qi * ICQ : (qi + 1) * ICQ].rearrange("ic p f -> p ic f"),
                )
                wpq[qi] = wq

            load_wp(0)
            xds0.append(load_xd(0, 0))
            load_wp(1)
            load_wp(2)
            xds0.append(load_xd(0, XDC))
            load_wp(3)
            load_wp(4)
            xds0.append(load_xd(0, 2 * XDC))
            load_wp(5)
            load_wp(6)
            xds0.append(load_xd(0, 3 * XDC))
            load_wp(7)

            # persistent logits [P, g, ic, h] fp16; mh bounce tile [P, 128]
            logits = lg_pool.tile([P, NGRP, IC, H], f16, tag="logits")
            mh128 = lg_pool.tile([P, P], f16, tag="mh128")
            nc.gpsimd.memset(mh128[:, GH:P], 0.0)

            # V = running sum of v (fp16), vfin = final f32 v
            V = vv_pool.tile([B8, NGRP, WD, H], f16, tag="V")
            vb16 = vv_pool.tile([P, NGRP, WD, H], f16, tag="vb16")
            sun = vv_pool.tile([B8, NGRP, 170], f32, tag="sun")

            us = []
            dump_m = [True]
            dump_p = [True]

            def ugen_and_s0(g):
                """u-gen for group g; streams s0 partial sums on the fly."""
                u = u_pool.tile([P, IC, WD, H], f16, tag="u")
                sps = psum_s.tile([B8, CPY, 170], f32, tag="sps")
                for xi, xc in enumerate(range(0, IC, XDC)):
                    xd = xds0[xi] if g == 0 else load_xd(g, xc)
                    for j in range(0, XDC, CPY):
                        ps = psum_u.tile([P, CPY, WH], f32, tag="ups")
                        for t in range(CPY):
                            ic = xc + j + t
                            nc.tensor.matmul(
                                ps[:, t, :],
                                xd[:, j + t, :],
                                wpq[ic // ICQ][:, ic % ICQ, :],
                                start=True,
                                stop=True,
                            )
                        ic0 = xc + j
                        nc.scalar.copy(
                            u[:, ic0 : ic0 + CPY],
                            ps[:].rearrange("p a (w h) -> p a w h", h=H),
                        )
                        # s0 partial: stream u chunk through sdelta matmul
                        nc.tensor.matmul(
                            sps[:, :, 0:WH],
                            sdelta[:],
                            u[:, ic0 : ic0 + CPY],
                            start=(ic0 == 0),
                            stop=(ic0 == IC - CPY),
                        )
                us.append(u)
                # fold the 3 partial columns -> sun[:, g]
                nc.vector.reduce_sum(
                    sun[:, g, 0:WH],
                    sps[:, :, 0:WH].rearrange("b a f -> b f a"),
                    axis=AX.X,
                )

            def squash(g_slice, it):
                """Squash sun -> v; writes V/vfin and returns v tile.
                g_slice: list of groups covered (all, fused)."""
                n = len(g_slice)
                g0 = g_slice[0]
                sw = sun[:, g0 : g0 + n, 0:WH].rearrange(
                    "b g (w h) -> b g w h", h=H
                )
                s = small_pool.tile([B8, n, WD, H], f32, tag="s")
                if it == 0:
                    nc.vector.tensor_scalar_mul(s[:], sw, 1.0 / NI)
                else:
                    dinv = small_pool.tile([B8, n, H], f32, tag="dinv")
                    nc.vector.reciprocal(
                        dinv[:],
                        sun[:, g0 : g0 + n, WH:170].rearrange(
                            "b g h -> b g h"
                        ),
                    )
                    nc.vector.tensor_mul(
                        s[:], sw, dinv[:].unsqueeze(2).to_broadcast([B8, n, WD, H])
                    )
                s2 = small_pool.tile([B8, n, WD, H], f32, tag="s2")
                nc.scalar.activation(s2[:], s[:], AF.Square)
                sq = small_pool.tile([B8, n, H], f32, tag="sq")
                nc.vector.reduce_sum(
                    sq[:], s2[:].rearrange("b g w h -> b g h w"), axis=AX.X
                )
                lgq = small_pool.tile([B8, n, H], f32, tag="lgq")
                nc.scalar.activation(lgq[:], sq[:], AF.Ln)
                rt = small_pool.tile([B8, n, H], f32, tag="rt")
                nc.scalar.activation(rt[:], lgq[:], AF.Exp, scale=0.5)
                onep = small_pool.tile([B8, n, H], f32, tag="onep")
                nc.vector.tensor_scalar_add(onep[:], sq[:], 1.0)
                rr = small_pool.tile([B8, n, H], f32, tag="rr")
                nc.vector.reciprocal(rr[:], onep[:])
                f = small_pool.tile([B8, n, H], f32, tag="f")
                nc.vector.tensor_mul(f[:], rt[:], rr[:])
                fb = f[:].unsqueeze(2).to_broadcast([B8, n, WD, H])
                if it == 2:
                    vfin = small_pool.tile([B8, n, WD, H], f32, tag="vfin")
                    nc.vector.tensor_mul(vfin[:], s[:], fb)
                    return vfin
                if it == 0:
                    # V slot(s) initialized directly
                    nc.vector.tensor_mul(V[:, g0 : g0 + n], s[:], fb)
                    return None
                v16 = small_pool.tile([B8, n, WD, H], f16, tag="v16")
                nc.vector.tensor_mul(v16[:], s[:], fb)
                nc.vector.tensor_add(V[:, g0 : g0 + n], V[:, g0 : g0 + n], v16[:])
                return None

            def vbcast(g_slice):
                """vb16[:, g] = broadcast of V[:, g] to all partitions."""
                for g in g_slice:
                    vbp = psum_v.tile([P, WD, H], f32, tag="vbp")
                    nc.tensor.matmul(
                        vbp[:], srepl[:], V[:, g], start=True, stop=True
                    )
                    nc.scalar.copy(vb16[:, g], vbp[:])

            HIC = IC // 2  # 36

            def apass(g, pe_fold=False):
                """logits[:, g] = sum_w u * vb16[:, g], in two ic-halves.
                pe_fold: fold on TensorE (16 accumulating identity matmuls
                into f32 PSUM + ScalarE copy) to fill PE idle windows;
                else in-place fp16 DVE fold tree."""
                u = us[g]
                for a in (0, HIC):
                    pra = pra_pool.tile([P, HIC, WD, H], f16, tag="pra")
                    vbb = vb16[:, g].unsqueeze(1).to_broadcast([P, HIC, WD, H])
                    nc.vector.tensor_mul(pra[:], u[:, a : a + HIC], vbb)
                    lslice = logits[:, g, a : a + HIC, :]
                    if pe_fold:
                        pa = psum_a.tile([P, HIC, H], f32, tag="pa")
                        for w in range(WD):
                            nc.tensor.matmul(
                                pa[:],
                                ident[:],
                                pra[:, :, w, :],
                                start=(w == 0),
                                stop=(w == WD - 1),
                            )
                        nc.scalar.copy(lslice, pa[:])
                    else:
                        nc.vector.tensor_add(
                            pra[:, :, 0:8, :],
                            pra[:, :, 0:8, :],
                            pra[:, :, 8:16, :],
                        )
                        nc.vector.tensor_add(
                            pra[:, :, 0:4, :],
                            pra[:, :, 0:4, :],
                            pra[:, :, 4:8, :],
                        )
                        nc.vector.tensor_add(
                            pra[:, :, 0:2, :],
                            pra[:, :, 0:2, :],
                            pra[:, :, 2:4, :],
                        )
                        nc.vector.tensor_add(
                            lslice, pra[:, :, 0, :], pra[:, :, 1, :]
                        )

            def maxshift_exp(cexp):
                """Fused over groups: per-(b,h) max, shift logits, exp."""
                mt = mx_pool.tile([P, NGRP, 36, H], f16, tag="mt")
                nc.vector.tensor_max(
                    mt[:], logits[:, :, 0:36, :], logits[:, :, 36:72, :]
                )
                nc.vector.tensor_max(
                    mt[:, :, 0:18, :], mt[:, :, 0:18, :], mt[:, :, 18:36, :]
                )
                nc.vector.tensor_max(
                    mt[:, :, 0:9, :], mt[:, :, 0:9, :], mt[:, :, 9:18, :]
                )
                nc.vector.reduce_max(
                    mh128[:, 0:GH].rearrange("p (g h) -> p g h", g=NGRP),
                    mt[:, :, 0:9, :].rearrange("p g i h -> p g h i"),
                    axis=AX.X,
                )
                mhT = mx_pool.tile([P, P], f16, tag="mhT")
                nc.sync.dma_start_transpose(mhT[:], mh128[:])
                # fold i16 (outer half of partition index) on 40 lanes
                mxs = mx_pool.tile([GH, 64], f16, tag="mxs")
                nc.vector.tensor_max(mxs[:], mhT[0:GH, 0:64], mhT[0:GH, 64:128])
                nc.vector.tensor_max(mxs[:, 0:32], mxs[:, 0:32], mxs[:, 32:64])
                nc.vector.tensor_max(mxs[:, 0:16], mxs[:, 0:16], mxs[:, 16:32])
                nc.vector.tensor_max(mxs[:, 0:8], mxs[:, 0:8], mxs[:, 8:16])
                # broadcast back: M16[p, (g,h)] = mxs[(g,h), p%8]
                mxb = mx_pool.tile([GH, I16, B8], f16, tag="mxb")
                nc.vector.tensor_copy(
                    mxb[:], mxs[:, 0:8].unsqueeze(1).to_broadcast([GH, I16, B8])
                )
                mps = psum_v.tile([P, GH], f32, tag="mps")
                nc.tensor.matmul(
                    mps[:],
                    mxb[:],
                    eye40[:],
                    start=True,
                    stop=True,
                )
                m16 = mx_pool.tile([P, NGRP, H], f16, tag="m16")
                nc.scalar.copy(m16[:], mps[:].rearrange("p (g h) -> p g h", g=NGRP))
                if dumps and dump_m[0]:
                    dump_m[0] = False
                    nc.sync.dma_start(
                        dbg_m[:], m16[:].rearrange("p g h -> p (g h)")
                    )
                nc.vector.tensor_sub(
                    logits[:],
                    logits[:],
                    m16[:].unsqueeze(2).to_broadcast([P, NGRP, IC, H]),
                )
                nc.scalar.activation(cexp[:], logits[:], AF.Exp)

            def spass(g, cexp):
                """sun[:, g] (incl. d in cols 160:170) from pr = u*c stream."""
                u = us[g]
                sps = psum_s.tile([B8, CPY, 170], f32, tag="sps")
                dps = psum_v.tile([B8, CPY, H], f32, tag="dps")
                for ck, c0 in enumerate(range(0, IC, CKS)):
                    pr = prs_pool.tile([P, CKS, WD, H], f16, tag="pr")
                    cb = (
                        cexp[:, g, c0 : c0 + CKS, :]
                        .unsqueeze(2)
                        .to_broadcast([P, CKS, WD, H])
                    )
                    nc.vector.tensor_mul(pr[:], u[:, c0 : c0 + CKS], cb)
                    if dumps and dump_p[0] and g == 0 and c0 == 0:
                        dump_p[0] = False
                        nc.sync.dma_start(
                            dbg_p[:], pr[:].rearrange("p ic w h -> p (ic w h)")
                        )
                    for j in range(0, CKS, CPY):
                        ic = c0 + j
                        nc.tensor.matmul(
                            sps[:, :, 0:WH],
                            sdelta[:],
                            pr[:, j : j + CPY],
                            start=(ic == 0),
                            stop=(ic == IC - CPY),
                        )
                        nc.tensor.matmul(
                            dps[:],
                            sdelta[:],
                            cexp[:, g, ic : ic + CPY, :],
                            start=(ic == 0),
                            stop=(ic == IC - CPY),
                        )
                nc.vector.reduce_sum(
                    sun[:, g, 0:WH],
                    sps[:, :, 0:WH].rearrange("b a f -> b f a"),
                    axis=AX.X,
                )
                nc.vector.reduce_sum(
                    sun[:, g, WH:170],
                    dps[:].rearrange("b a h -> b h a"),
                    axis=AX.X,
                )

            # ================= iteration 0 =================
            for g in range(NGRP):
                ugen_and_s0(g)
                squash([g], 0)
                vbcast([g])
                apass(g, pe_fold=(g == NGRP - 1))

            if dumps:
                nc.sync.dma_start(
                    dbg_u[:], us[0][:].rearrange("p ic w h -> p (ic w h)")
                )
                nc.sync.dma_start(
                    dbg_l[:], logits[:].rearrange("p g ic h -> p (g ic h)")
                )

            # ================= iterations 1, 2 =================
            for it in (1, 2):
                cexp = c_pool.tile([P, NGRP, IC, H], f16, tag="cexp")
                maxshift_exp(cexp)
                if dumps and it == 1:
                    nc.sync.dma_start(
                        dbg_c[:], cexp[:].rearrange("p g ic h -> p (g ic h)")
                    )
                for g in range(NGRP):
                    spass(g, cexp)
                    if it == 2:
                        vfin = squash([g], 2)
                        nc.sync.dma_start(
                            out_d[g * B8 : (g + 1) * B8, :],
                            vfin[:].rearrange("b g w h -> b (g w h)"),
                        )
                if dumps and it == 1:
                    nc.sync.dma_start(
                        dbg_s[:], sun[:].rearrange("b g f -> b (g f)")
                    )
                if it == 1:
                    squash(list(range(NGRP)), 1)
                    if dumps:
                        nc.sync.dma_start(
                            dbg_v[:], V[:].rearrange("b g w h -> b (g w h)")
                        )
                    vbcast(list(range(NGRP)))
                    for g in range(NGRP):
                        apass(g, pe_fold=(g < NGRP - 1))
                    if dumps:
                        nc.sync.dma_start(
                            dbg_l2[:], logits[:].rearrange("p g ic h -> p (g ic h)")
                        )
                elif dumps:
                    nc.sync.dma_start(
                        dbg_s2[:], sun[:].rearrange("b g f -> b (g f)")
                    )

    nc.compile()
    return nc


def _host_inputs(x: np.ndarray, W: np.ndarray):
    """Build per-core input maps."""
    xr = np.ascontiguousarray(x.reshape(B_FULL, NI, S).astype(np.float32, copy=False))
    W0 = np.asarray(W, dtype=np.float32).reshape(H, NI, WD, S)
    # wpack[ic, (i16,s), (w,h)] = W0[h, ic*16+i16, w, s]
    wpack = np.ascontiguousarray(
        W0.reshape(H, IC, I16, WD, S)
        .transpose(1, 2, 4, 3, 0)
        .reshape(IC, P, WH)
        .astype(np.float16)
    )
    # sdelta[p, b'] = (p % 8 == b');  srepl = sdelta.T
    pidx = np.arange(P)
    sdelta = (pidx[:, None] % B8 == np.arange(B8)[None, :]).astype(np.float16)
    srepl = np.ascontiguousarray(sdelta.T)
    eye40 = np.eye(GH, dtype=np.float16)
    ident = np.eye(P, dtype=np.float16)

    in_maps = []
    for c in range(N_CORES):
        xc = xr[c * B_CORE : (c + 1) * B_CORE]  # [32, 1152, 8]
        # xdiag[g, (i16,s), ic*128 + i16*8 + b] = xc[g*8+b, ic*16+i16, s]
        xd = np.zeros((NGRP, P, IC, I16, B8), dtype=np.float16)
        xg = xc.reshape(NGRP, B8, IC, I16, S).astype(np.float16)
        for k in range(I16):
            xd[:, k * S : (k + 1) * S, :, k, :] = xg[:, :, :, k, :].transpose(
                0, 3, 2, 1
            )
        in_maps.append(
            {
                "xdiag": np.ascontiguousarray(xd.reshape(NGRP, P, IC * P)),
                "wpack": wpack,
                "sdelta": sdelta,
                "srepl": srepl,
                "eye40": eye40,
                "ident": ident,
            }
        )
    return in_maps


def _unshard(vout: np.ndarray) -> np.ndarray:
    """Per-core vout [B_CORE, (w,h)] -> [B_CORE, H, WD]."""
    return vout.reshape(B_CORE, WD, H).transpose(0, 2, 1)


def kernel(x: np.ndarray, W: np.ndarray) -> np.ndarray:
    from concourse import bass_utils

    if "nc" not in _CACHE:
        _CACHE["nc"] = _build_program(debug=False)
    nc = _CACHE["nc"]
    in_maps = _host_inputs(x, W)
    res = bass_utils.run_bass_kernel_spmd(nc, in_maps, list(range(N_CORES)))
    outs = [_unshard(res.results[c]["vout"]) for c in range(N_CORES)]
    return np.concatenate(outs, axis=0).astype(np.float32)


# revision 35
# speedup vs baseline: 1.1680x; 1.0322x over previous
"""DigitCaps (capsule routing) Trainium2 kernel, v1 (h-inner layout).

Self-contained: hardcodes shapes for
  x: [256, 32, 8, 6, 6] f32, W: [1, 10, 1152, 16, 8] f32 -> v: [256, 10, 16] f32

Sharding: pure data parallelism over batch, 32 batch items per core on 8
cores, processed as 4 octet groups per core.

Per-core layout: partition p = (i16, b8); u free dims ordered (ic=72,
w=16, h=10) with h INNERMOST so both big DVE muls run in fp16 2x mode
with no operand materialization:
  - s-pass: pr = u * c with c[p,ic,h] broadcast along w (middle axis);
  - a-pass: pr = u * vb with vb[p,w,h] broadcast along ic (outer axis).
u = W@x from block-diag packed fp16 matmuls (K=128: lhsT = host-built
block-diag x tile, rhs = repacked W, w-major/h-minor free order).
Logits are fp16, produced directly by in-place pairwise fold trees over w
(all 2x); l(t) = U.(v0+..+v_{t-1}) is recomputed fresh each iteration from
the running V so there is no read-modify-write on logits.  Softmax safety
shift: per-(b,h) max via fp16 max-fold tree over ic, a DMA xbar transpose
to fold i16 across partitions, and a tiny PE matmul (lhsT = max-bcast
view, rhs = eye40) to broadcast the per-(b,h) max back to all partitions.
The softmax denominator d = sum_i c accumulates in spare PSUM columns
(160:170) of the s-reduction tile by streaming cexp chunks through the
same sdelta matmul.  sqrt in squash is exp(0.5*ln(x)) so ACT stays on one
table set.  Output rows are (w,h)-ordered; the host transposes.
"""

import numpy as np

# ---- problem constants (hardcoded) ----
B_FULL = 256
N_CORES = 8
B_CORE = B_FULL // N_CORES          # 32
NGRP = 4                            # octet groups per core
B8 = 8                              # batch per group
H = 10
WD = 16
WH = WD * H                         # 160
S = 8
NI = 1152
I16 = 16
IC = NI // I16                      # 72
ICQ = 9                             # ic per wpack DMA chunk
XDC = 18                            # ic per xdiag DMA chunk
CPY = 3                             # ic per u psum copy tile
CKS = 36                            # ic per s-pass mul chunk
P = 128
GH = NGRP * H                       # 40

_CACHE = {}


def _build_program(debug: bool, dumps: bool = False):
    import concourse.bacc as bacc
    import concourse.bass as bass
    import concourse.tile as tile
    from concourse import mybir

    f32 = mybir.dt.float32
    f16 = mybir.dt.float16
    AX = mybir.AxisListType
    AF = mybir.ActivationFunctionType

    if not getattr(bacc, "_digitcaps_act_pin", False):
        _orig_gat = bacc.get_activation_tables

        def _pinned_gat(arch):
            tables = dict(_orig_gat(arch))
            both = {mybir.ActivationFunctionType.Exp, mybir.ActivationFunctionType.Ln}
            for name in tables:
                if name != "natural_log_exp_and_others" and both & tables[name]:
                    tables[name] = tables[name] - both
            return tables

        bacc.get_activation_tables = _pinned_gat
        bacc._digitcaps_act_pin = True

    nc = bacc.Bacc(
        "TRN2", target_bir_lowering=False, debug=debug, enable_asserts=False
    )

    xd_d = nc.dram_tensor("xdiag", [NGRP, P, IC * P], f16, kind="ExternalInput")
    w_d = nc.dram_tensor("wpack", [P, IC * WH], f16, kind="ExternalInput")
    sd_d = nc.dram_tensor("sdelta", [P, B8], f16, kind="ExternalInput")
    sr_d = nc.dram_tensor("srepl", [B8, P], f16, kind="ExternalInput")
    ey_d = nc.dram_tensor("eye40", [GH, GH], f16, kind="ExternalInput")
    id_d = nc.dram_tensor("ident", [P, P], f16, kind="ExternalInput")
    out_d = nc.dram_tensor("vout", [B_CORE, WH], f32, kind="ExternalOutput")
    if dumps:
        dbg_u = nc.dram_tensor("dbg_u", [P, IC * WH], f16, kind="ExternalOutput")
        dbg_l = nc.dram_tensor("dbg_l", [P, NGRP * IC * H], f16, kind="ExternalOutput")
        dbg_m = nc.dram_tensor("dbg_m", [P, GH], f16, kind="ExternalOutput")
        dbg_c = nc.dram_tensor("dbg_c", [P, NGRP * IC * H], f16, kind="ExternalOutput")
        dbg_s = nc.dram_tensor("dbg_s", [B8, NGRP * 170], f32, kind="ExternalOutput")
        dbg_p = nc.dram_tensor("dbg_p", [P, CKS * WH], f16, kind="ExternalOutput")
        dbg_l2 = nc.dram_tensor("dbg_l2", [P, NGRP * IC * H], f16, kind="ExternalOutput")
        dbg_s2 = nc.dram_tensor("dbg_s2", [B8, NGRP * 170], f32, kind="ExternalOutput")
        dbg_v = nc.dram_tensor("dbg_v", [B8, NGRP * WH], f16, kind="ExternalOutput")

    with tile.TileContext(nc) as tc:
        with (
            tc.tile_pool(name="const", bufs=1) as const_pool,
            tc.tile_pool(name="wp", bufs=1) as wp_pool,
            tc.tile_pool(name="xd", bufs=2) as xd_pool,
            tc.tile_pool(name="u", bufs=4) as u_pool,
            tc.tile_pool(name="prs", bufs=2) as prs_pool,
            tc.tile_pool(name="pra", bufs=2) as pra_pool,
            tc.tile_pool(name="lg", bufs=1) as lg_pool,
            tc.tile_pool(name="cexp", bufs=1) as c_pool,
            tc.tile_pool(name="mx", bufs=1) as mx_pool,
            tc.tile_pool(name="small", bufs=2) as small_pool,
            tc.tile_pool(name="vv", bufs=1) as vv_pool,
            tc.tile_pool(name="psum_u", bufs=2, space="PSUM") as psum_u,
            tc.tile_pool(name="psum_s", bufs=2, space="PSUM") as psum_s,
            tc.tile_pool(name="psum_v", bufs=1, space="PSUM") as psum_v,
            tc.tile_pool(name="psum_a", bufs=1, space="PSUM") as psum_a,
        ):
            sdelta = const_pool.tile([P, B8], f16, tag="sdelta")
            nc.sync.dma_start(sdelta[:], sd_d[:])
            srepl = const_pool.tile([B8, P], f16, tag="srepl")
            nc.sync.dma_start(srepl[:], sr_d[:])
            eye40 = const_pool.tile([GH, GH], f16, tag="eye40")
            nc.sync.dma_start(eye40[:], ey_d[:])
            ident = const_pool.tile([P, P], f16, tag="ident")
            nc.sync.dma_start(ident[:], id_d[:])

            def load_xd(g, xc):
                xd = xd_pool.tile([P, XDC, P], f16, tag="xd")
                nc.sync.dma_start(
                    xd[:],
                    xd_d[g].rearrange("p (ic m) -> p ic m", ic=IC)[
                        :, xc : xc + XDC
                    ],
                )
                return xd

            # resident W pack, split per chunk so deps are chunk-granular.
            # Emission interleaves group 0's xd loads between wpack chunks so
            # the first u-gen matmul isn't stuck behind the whole W transfer
            # on the serial sync DMA queue.
            wpq = [None] * (IC // ICQ)
            xds0 = []

            def load_wp(qi):
                wq = wp_pool.tile([P, ICQ, WH], f16, tag=f"wp{qi}")
                nc.sync.dma_start(
                    wq[:],
                    w_d[:].rearrange("p (ic f) -> p ic f", f=WH)[
                        :, qi * ICQ : (qi + 1) * ICQ
                    ],
                )
                wpq[qi] = wq

            load_wp(0)
            xds0.append(load_xd(0, 0))
            load_wp(1)
            load_wp(2)
            xds0.append(load_xd(0, XDC))
            load_wp(3)
            load_wp(4)
            xds0.append(load_xd(0, 2 * XDC))
            load_wp(5)
            load_wp(6)
            xds0.append(load_xd(0, 3 * XDC))
            load_wp(7)

            # persistent logits [P, g, ic, h] fp16; mh bounce tile [P, 128]
            logits = lg_pool.tile([P, NGRP, IC, H], f16, tag="logits")
            mh128 = lg_pool.tile([P, P], f16, tag="mh128")
            nc.gpsimd.memset(mh128[:, GH:P], 0.0)

            # V = running sum of v (fp16), vfin = final f32 v
            V = vv_pool.tile([B8, NGRP, WD, H], f16, tag="V")
            vb16 = vv_pool.tile([P, NGRP, WD, H], f16, tag="vb16")
            sun = vv_pool.tile([B8, NGRP, 170], f32, tag="sun")

            us = []
            dump_m = [True]
            dump_p = [True]

            def ugen_and_s0(g):
                """u-gen for group g; streams s0 partial sums on the fly."""
                u = u_pool.tile([P, IC, WD, H], f16, tag="u")
                sps = psum_s.tile([B8, CPY, 170], f32, tag="sps")
                for xi, xc in enumerate(range(0, IC, XDC)):
                    xd = xds0[xi] if g == 0 else load_xd(g, xc)
                    for j in range(0, XDC, CPY):
                        ps = psum_u.tile([P, CPY, WH], f32, tag="ups")
                        for t in range(CPY):
                            ic = xc + j + t
                            nc.tensor.matmul(
                                ps[:, t, :],
                                xd[:, j + t, :],
                                wpq[ic // ICQ][:, ic % ICQ, :],
                                start=True,
                                stop=True,
                            )
                        ic0 = xc + j
                        nc.scalar.copy(
                            u[:, ic0 : ic0 + CPY],
                            ps[:].rearrange("p a (w h) -> p a w h", h=H),
                        )
                        # s0 partial: stream u chunk through sdelta matmul
                        nc.tensor.matmul(
                            sps[:, :, 0:WH],
                            sdelta[:],
                            u[:, ic0 : ic0 + CPY],
                            start=(ic0 == 0),
                            stop=(ic0 == IC - CPY),
                        )
                us.append(u)
                # fold the 3 partial columns -> sun[:, g]
                nc.vector.reduce_sum(
                    sun[:, g, 0:WH],
                    sps[:, :, 0:WH].rearrange("b a f -> b f a"),
                    axis=AX.X,
                )

            def squash(g_slice, it):
                """Squash sun -> v; writes V/vfin and returns v tile.
                g_slice: list of groups covered (all, fused)."""
                n = len(g_slice)
                g0 = g_slice[0]
                sw = sun[:, g0 : g0 + n, 0:WH].rearrange(
                    "b g (w h) -> b g w h", h=H
                )
                s = small_pool.tile([B8, n, WD, H], f32, tag="s")
                if it == 0:
                    nc.vector.tensor_scalar_mul(s[:], sw, 1.0 / NI)
                else:
                    dinv = small_pool.tile([B8, n, H], f32, tag="dinv")
                    nc.vector.reciprocal(
                        dinv[:],
                        sun[:, g0 : g0 + n, WH:170].rearrange(
                            "b g h -> b g h"
                        ),
                    )
                    nc.vector.tensor_mul(
                        s[:], sw, dinv[:].unsqueeze(2).to_broadcast([B8, n, WD, H])
                    )
                s2 = small_pool.tile([B8, n, WD, H], f32, tag="s2")
                nc.scalar.activation(s2[:], s[:], AF.Square)
                sq = small_pool.tile([B8, n, H], f32, tag="sq")
                nc.vector.reduce_sum(
                    sq[:], s2[:].rearrange("b g w h -> b g h w"), axis=AX.X
                )
                lgq = small_pool.tile([B8, n, H], f32, tag="lgq")
                nc.scalar.activation(lgq[:], sq[:], AF.Ln)
                rt = small_pool.tile([B8, n, H], f32, tag="rt")
                nc.scalar.activation(rt[:], lgq[:], AF.Exp, scale=0.5)
                onep = small_pool.tile([B8, n, H], f32, tag="onep")
                nc.vector.tensor_scalar_add(onep[:], sq[:], 1.0)
                rr = small_pool.tile([B8, n, H], f32, tag="rr")
                nc.vector.reciprocal(rr[:], onep[:])
                f = small_pool.tile([B8, n, H], f32, tag="f")
                nc.vector.tensor_mul(f[:], rt[:], rr[:])
                fb = f[:].unsqueeze(2).to_broadcast([B8, n, WD, H])
                if it == 2:
                    vfin = small_pool.tile([B8, n, WD, H], f32, tag="vfin")
                    nc.vector.tensor_mul(vfin[:], s[:], fb)
                    return vfin
                if it == 0:
                    # V slot(s) initialized directly
                    nc.vector.tensor_mul(V[:, g0 : g0 + n], s[:], fb)
                    return None
                v16 = small_pool.tile([B8, n, WD, H], f16, tag="v16")
                nc.vector.tensor_mul(v16[:], s[:], fb)
                nc.vector.tensor_add(V[:, g0 : g0 + n], V[:, g0 : g0 + n], v16[:])
                return None

            def vbcast(g_slice):
                """vb16[:, g] = broadcast of V[:, g] to all partitions."""
                for g in g_slice:
                    vbp = psum_v.tile([P, WD, H], f32, tag="vbp")
                    nc.tensor.matmul(
                        vbp[:], srepl[:], V[:, g], start=True, stop=True
                    )
                    nc.scalar.copy(vb16[:, g], vbp[:])

            HIC = IC // 2  # 36

            def apass(g, pe_fold=False):
                """logits[:, g] = sum_w u * vb16[:, g], in two ic-halves.
                pe_fold: fold on TensorE (16 accumulating identity matmuls
                into f32 PSUM + ScalarE copy) to fill PE idle windows;
                else in-place fp16 DVE fold tree."""
                u = us[g]
                for a in (0, HIC):
                    pra = pra_pool.tile([P, HIC, WD, H], f16, tag="pra")
                    vbb = vb16[:, g].unsqueeze(1).to_broadcast([P, HIC, WD, H])
                    nc.vector.tensor_mul(pra[:], u[:, a : a + HIC], vbb)
                    lslice = logits[:, g, a : a + HIC, :]
                    if pe_fold:
                        pa = psum_a.tile([P, HIC, H], f32, tag="pa")
                        for w in range(WD):
                            nc.tensor.matmul(
                                pa[:],
                                ident[:],
                                pra[:, :, w, :],
                                start=(w == 0),
                                stop=(w == WD - 1),
                            )
                        nc.scalar.copy(lslice, pa[:])
                    else:
                        nc.vector.tensor_add(
                            pra[:, :, 0:8, :],
                            pra[:, :, 0:8, :],
                            pra[:, :, 8:16, :],
                        )
                        nc.vector.tensor_add(
                            pra[:, :, 0:4, :],
                            pra[:, :, 0:4, :],
                            pra[:, :, 4:8, :],
                        )
                        nc.vector.tensor_add(
                            pra[:, :, 0:2, :],
                            pra[:, :, 0:2, :],
                            pra[:, :, 2:4, :],
                        )
                        nc.vector.tensor_add(
                            lslice, pra[:, :, 0, :], pra[:, :, 1, :]
                        )

            def maxshift_exp(cexp):
                """Fused over groups: per-(b,h) max, shift logits, exp."""
                mt = mx_pool.tile([P, NGRP, 36, H], f16, tag="mt")
                nc.vector.tensor_max(
                    mt[:], logits[:, :, 0:36, :], logits[:, :, 36:72, :]
                )
                nc.vector.tensor_max(
                    mt[:, :, 0:18, :], mt[:, :, 0:18, :], mt[:, :, 18:36, :]
                )
                nc.vector.tensor_max(
                    mt[:, :, 0:9, :], mt[:, :, 0:9, :], mt[:, :, 9:18, :]
                )
                nc.vector.reduce_max(
                    mh128[:, 0:GH].rearrange("p (g h) -> p g h", g=NGRP),
                    mt[:, :, 0:9, :].rearrange("p g i h -> p g h i"),
                    axis=AX.X,
                )
                mhT = mx_pool.tile([P, P], f16, tag="mhT")
                nc.sync.dma_start_transpose(mhT[:], mh128[:])
                # fold i16 (outer half of partition index) on 40 lanes
                mxs = mx_pool.tile([GH, 64], f16, tag="mxs")
                nc.vector.tensor_max(mxs[:], mhT[0:GH, 0:64], mhT[0:GH, 64:128])
                nc.vector.tensor_max(mxs[:, 0:32], mxs[:, 0:32], mxs[:, 32:64])
                nc.vector.tensor_max(mxs[:, 0:16], mxs[:, 0:16], mxs[:, 16:32])
                nc.vector.tensor_max(mxs[:, 0:8], mxs[:, 0:8], mxs[:, 8:16])
                # broadcast back: M16[p, (g,h)] = mxs[(g,h), p%8]
                mxb = mx_pool.tile([GH, I16, B8], f16, tag="mxb")
                nc.vector.tensor_copy(
                    mxb[:], mxs[:, 0:8].unsqueeze(1).to_broadcast([GH, I16, B8])
                )
                mps = psum_v.tile([P, GH], f32, tag="mps")
                nc.tensor.matmul(
                    mps[:],
                    mxb[:],
                    eye40[:],
                    start=True,
                    stop=True,
                )
                m16 = mx_pool.tile([P, NGRP, H], f16, tag="m16")
                nc.scalar.copy(m16[:], mps[:].rearrange("p (g h) -> p g h", g=NGRP))
                if dumps and dump_m[0]:
                    dump_m[0] = False
                    nc.sync.dma_start(
                        dbg_m[:], m16[:].rearrange("p g h -> p (g h)")
                    )
                nc.vector.tensor_sub(
                    logits[:],
                    logits[:],
                    m16[:].unsqueeze(2).to_broadcast([P, NGRP, IC, H]),
                )
                nc.scalar.activation(cexp[:], logits[:], AF.Exp)

            def spass(g, cexp):
                """sun[:, g] (incl. d in cols 160:170) from pr = u*c stream."""
                u = us[g]
                sps = psum_s.tile([B8, CPY, 170], f32, tag="sps")
                dps = psum_v.tile([B8, 12, H], f32, tag="dps")
                for ck, c0 in enumerate(range(0, IC, CKS)):
                    pr = prs_pool.tile([P, CKS, WD, H], f16, tag="pr")
                    cb = (
                        cexp[:, g, c0 : c0 + CKS, :]
                        .unsqueeze(2)
                        .to_broadcast([P, CKS, WD, H])
                    )
                    nc.vector.tensor_mul(pr[:], u[:, c0 : c0 + CKS], cb)
                    for j12 in range(0, CKS, 12):
                        ic12 = c0 + j12
                        nc.tensor.matmul(
                            dps[:],
                            sdelta[:],
                            cexp[:, g, ic12 : ic12 + 12, :],
                            start=(ic12 == 0),
                            stop=(ic12 == IC - 12),
                        )
                    if dumps and dump_p[0] and g == 0 and c0 == 0:
                        dump_p[0] = False
                        nc.sync.dma_start(
                            dbg_p[:], pr[:].rearrange("p ic w h -> p (ic w h)")
                        )
                    for j in range(0, CKS, CPY):
                        ic = c0 + j
                        nc.tensor.matmul(
                            sps[:, :, 0:WH],
                            sdelta[:],
                            pr[:, j : j + CPY],
                            start=(ic == 0),
                            stop=(ic == IC - CPY),
                        )

                nc.vector.reduce_sum(
                    sun[:, g, 0:WH],
                    sps[:, :, 0:WH].rearrange("b a f -> b f a"),
                    axis=AX.X,
                )
                nc.vector.reduce_sum(
                    sun[:, g, WH:170],
                    dps[:].rearrange("b a h -> b h a"),
                    axis=AX.X,
                )

            # ================= iteration 0 =================
            for g in range(NGRP):
                ugen_and_s0(g)
                squash([g], 0)
                vbcast([g])
                apass(g)

            if dumps:
                nc.sync.dma_start(
                    dbg_u[:], us[0][:].rearrange("p ic w h -> p (ic w h)")
                )
                nc.sync.dma_start(
                    dbg_l[:], logits[:].rearrange("p g ic h -> p (g ic h)")
                )

            # ================= iterations 1, 2 =================
            for it in (1, 2):
                cexp = c_pool.tile([P, NGRP, IC, H], f16, tag="cexp")
                maxshift_exp(cexp)
                if dumps and it == 1:
                    nc.sync.dma_start(
                        dbg_c[:], cexp[:].rearrange("p g ic h -> p (g ic h)")
                    )
                for g in range(NGRP):
                    spass(g, cexp)
                    if it == 2:
                        vfin = squash([g], 2)
                        nc.sync.dma_start(
                            out_d[g * B8 : (g + 1) * B8, :],
                            vfin[:].rearrange("b g w h -> b (g w h)"),
                        )
                if dumps and it == 1:
                    nc.sync.dma_start(
                        dbg_s[:], sun[:].rearrange("b g f -> b (g f)")
                    )
                if it == 1:
                    squash(list(range(NGRP)), 1)
                    if dumps:
                        nc.sync.dma_start(
                            dbg_v[:], V[:].rearrange("b g w h -> b (g w h)")
                        )
                    vbcast(list(range(NGRP)))
                    for g in range(NGRP):
                        apass(g, pe_fold=(g < NGRP - 1))
                    if dumps:
                        nc.sync.dma_start(
                            dbg_l2[:], logits[:].rearrange("p g ic h -> p (g ic h)")
                        )
                elif dumps:
                    nc.sync.dma_start(
                        dbg_s2[:], sun[:].rearrange("b g f -> b (g f)")
                    )

    nc.compile()
    return nc


def _host_inputs(x: np.ndarray, W: np.ndarray):
    """Build per-core input maps."""
    xr = np.ascontiguousarray(x.reshape(B_FULL, NI, S).astype(np.float32, copy=False))
    W0 = np.asarray(W, dtype=np.float32).reshape(H, NI, WD, S)
    # wpack[ic, (i16,s), (w,h)] = W0[h, ic*16+i16, w, s]
    wpack = np.ascontiguousarray(
        W0.reshape(H, IC, I16, WD, S)
        .transpose(2, 4, 1, 3, 0)
        .reshape(P, IC * WH)
        .astype(np.float16)
    )
    # sdelta[p, b'] = (p % 8 == b');  srepl = sdelta.T
    pidx = np.arange(P)
    sdelta = (pidx[:, None] % B8 == np.arange(B8)[None, :]).astype(np.float16)
    srepl = np.ascontiguousarray(sdelta.T)
    eye40 = np.eye(GH, dtype=np.float16)
    ident = np.eye(P, dtype=np.float16)

    in_maps = []
    for c in range(N_CORES):
        xc = xr[c * B_CORE : (c + 1) * B_CORE]  # [32, 1152, 8]
        # xdiag[g, (i16,s), ic*128 + i16*8 + b] = xc[g*8+b, ic*16+i16, s]
        xd = np.zeros((NGRP, P, IC, I16, B8), dtype=np.float16)
        xg = xc.reshape(NGRP, B8, IC, I16, S).astype(np.float16)
        for k in range(I16):
            xd[:, k * S : (k + 1) * S, :, k, :] = xg[:, :, :, k, :].transpose(
                0, 3, 2, 1
            )
        in_maps.append(
            {
                "xdiag": np.ascontiguousarray(xd.reshape(NGRP, P, IC * P)),
                "wpack": wpack,
                "sdelta": sdelta,
                "srepl": srepl,
                "eye40": eye40,
                "ident": ident,
            }
        )
    return in_maps


def _unshard(vout: np.ndarray) -> np.ndarray:
    """Per-core vout [B_CORE, (w,h)] -> [B_CORE, H, WD]."""
    return vout.reshape(B_CORE, WD, H).transpose(0, 2, 1)


def kernel(x: np.ndarray, W: np.ndarray) -> np.ndarray:
    from concourse import bass_utils

    if "nc" not in _CACHE:
        _CACHE["nc"] = _build_program(debug=False)
    nc = _CACHE["nc"]
    in_maps = _host_inputs(x, W)
    res = bass_utils.run_bass_kernel_spmd(nc, in_maps, list(range(N_CORES)))
    outs = [_unshard(res.results[c]["vout"]) for c in range(N_CORES)]
    return np.concatenate(outs, axis=0).astype(np.float32)
